# revision 1
# baseline (speedup 1.0000x reference)
"""Cross-attention layer (vision<->text) on 8 Trainium2 NeuronCores.

Problem: B=16, Sv=St=1024, D=1024, fp32.
  q = vision @ Wq.T + bq            [B,Sv,D]
  k = text   @ Wk.T + bk            [B,St,D]
  v = text   @ Wv.T + bv            [B,St,D]
  scores = q @ k.T / sqrt(D)        [B,Sv,St]
  attn = softmax(scores, -1)
  cross_vision = attn @ v           [B,Sv,D]
  cross_text   = attn.T @ vision    [B,St,D]

Sharding: pure data-parallel over batch, 2 items per core, no collectives.

Per-core kernel design (per batch item):
  - Host pre-transposes weights: wqt = Wq.T/sqrt(D) [d,e], wkt = Wk.T, wvt = Wv.T.
    The 1/sqrt(D) is folded into wqt/bq. bv is added on the host after gather
    (attn rows sum to 1, so attn @ (v0 + bv) = attn @ v0 + bv, exact).
  - On-chip PE transposes build VT[d,s] and TT[d,t] from the natural-layout
    activations, half the seq dim at a time (SBUF economy).
  - QT[e,s] = wqt.T @ VT, KT[e,t] = wkt.T @ TT (weight tile stationary),
    Vv[t,d'] = TT.T @ wvt (TT tile stationary). All matmuls run as float32r
    (fp32 bits, PE truncates to ~fp22: full-rate at N>=512, ~2^-12 rel err).
  - S[s,t] = QT.T @ KT per 128-row s-tile; E = exp(S) straight out of PSUM on
    the ACT engine with accum_out producing row sums (scores are O(+-6), no
    max-subtraction needed for fp32 exp). rinv = 1/rowsum.
  - cross_vision s-tile: PE-transpose E row-block -> ET blocks, then
    CV = ET.T @ Vv accumulated over t-tiles, scaled by rinv at PSUM evac.
  - E is then scaled in-place by rinv (making attn rows), and
    cross_text = E.T @ V accumulated over s-tiles with raw V streamed back in.
"""

import sys

import numpy as np

if "/opt/trn_rl_repo" not in sys.path:
    sys.path.insert(0, "/opt/trn_rl_repo")

import concourse.bass as bass
import concourse.tile as tile
from concourse import bacc
from concourse import mybir

PHASE_MARKS = []  # (phase_name, first_unused_instruction_id) at each boundary

P = 128
B, SEQ, DIM = 16, 1024, 1024
N_CORES = 8
BPC = B // N_CORES  # batch items per core
NT = DIM // P  # 8 tiles of 128 along d/e
F32 = mybir.dt.float32
F32R = mybir.dt.float32r
AF = mybir.ActivationFunctionType
H = 512  # half of a seq dim / PSUM-bank-sized chunk


def _emit(tc, ident, vis, txt, wqt, wkt, wvt, bq_sb, bk_sb, cv_d, ct_d, pools, b):
    nc = tc.nc

    def mark(name):
        nid = nc._state.next_id()
        PHASE_MARKS.append((f"b{b}_{name}", nid))

    (p_act, p_kt, p_qt, p_vv, p_etb, p_wc, p_vvt, p_in, p_cvs, p_cts, p_vt,
     p_rp, p_rv, pp_t, pp_mm) = pools

    kt = p_kt.tile([P, NT, SEQ], F32R, name="kt", tag="kt")
    vv = p_vv.tile([P, NT, SEQ], F32R, name="vv", tag="vv")
    qt = p_qt.tile([P, NT, SEQ], F32R, name="qt", tag="qt")

    def prep(src_d):
        """Transpose the full [SEQ, DIM] tensor into actT[d_in, d_out, seq].

        actT shares its pool slot with e_sb (disjoint lifetimes within an
        item: actT dies after projQ, e_sb is born in phase F).
        """
        actT = p_act.tile([P, NT, SEQ], F32R, name="actT", tag="act_e")
        for l in range(NT):
            for hh in range(2):  # two [128, 512] half-row loads, dual queue
                tin = p_in.tile([P, H], F32R, name="tin", tag="xin")
                eng = nc.sync if hh == 0 else nc.scalar
                eng.dma_start(
                    out=tin,
                    in_=src_d[b, l * P:(l + 1) * P, hh * H:(hh + 1) * H].bitcast(F32R))
                tp4 = pp_t.tile([P, 4, P], F32R, name="tp4", tag="tp4")
                for j in range(4):
                    do = hh * 4 + j
                    nc.tensor.matmul(
                        tp4[:, j, :], tin[:, j * P:(j + 1) * P], ident,
                        is_transpose=True, start=(j == 0), stop=(j == 3),
                        skip_group_check=True,
                    )
                if hh == 0:
                    nc.vector.tensor_copy(actT[:, 0:4, l * P:(l + 1) * P], tp4)
                else:
                    nc.scalar.copy(actT[:, 4:8, l * P:(l + 1) * P], tp4)
        return actT

    def proj(w_d, bias_col, actT, out_sb, on_vector):
        """out_sb[e_in, eo, s] = sum_do w[do,eo].T @ actT[:, do, :] (+bias).

        One 512KB weight-column load per eo (weight read once per item),
        16 matmuls per load across the two seq halves (2 PSUM groups).
        """
        for eo in range(NT):
            wc = p_wc.tile([P, NT, P], F32R, name="wc", tag="wc")
            nc.gpsimd.dma_start(
                out=wc,
                in_=w_d[:, eo * P:(eo + 1) * P].rearrange("(do di) e -> di do e", di=P),
            )
            pss = [pp_mm.tile([P, H], F32, name=f"ps_p{i}", tag="mm") for i in range(2)]
            for do in range(NT):
                for sh in range(2):
                    nc.tensor.matmul(pss[sh], wc[:, do, :], actT[:, do, sh * H:(sh + 1) * H],
                                     start=(do == 0), stop=(do == NT - 1))
            for sh in range(2):
                dst = out_sb[:, eo, sh * H:(sh + 1) * H]
                if on_vector:
                    nc.vector.tensor_scalar_add(dst, pss[sh], scalar1=bias_col[:, eo:eo + 1])
                else:
                    nc.scalar.add(dst, pss[sh], add=bias_col[:, eo:eo + 1])

    def proj_v(actT):
        """vv[t_in, tb, d'] = (TT.T @ wvt) via VvT then PE-transpose.

        VvT[d'-block, t] is computed with the weight columns stationary (one
        512KB load per d'-block, 16 matmuls each => Wv read once per item),
        evacuated to a small staging tile, then transposed 128x128-wise into
        the Vv[t, d'] layout cross_vision needs.
        """
        for dpo in range(NT):
            wvc = p_wc.tile([P, NT, P], F32R, name="wvc", tag="wc")
            nc.gpsimd.dma_start(
                out=wvc,
                in_=wvt[:, dpo * P:(dpo + 1) * P].rearrange("(do di) e -> di do e", di=P),
            )
            pss = [pp_mm.tile([P, H], F32, name=f"ps_v{i}", tag="mm") for i in range(2)]
            for do in range(NT):
                for th in range(2):
                    nc.tensor.matmul(pss[th], wvc[:, do, :], actT[:, do, th * H:(th + 1) * H],
                                     start=(do == 0), stop=(do == NT - 1))
            vvt_tmp = p_vvt.tile([P, SEQ], F32R, name="vvt_tmp", tag="vvt")
            for th in range(2):
                nc.scalar.copy(vvt_tmp[:, th * H:(th + 1) * H], pss[th])
            for tg in range(2):
                tp4 = pp_t.tile([P, 4, P], F32R, name="tp4v", tag="tp4")
                for j in range(4):
                    tb = tg * 4 + j
                    nc.tensor.matmul(tp4[:, j, :], vvt_tmp[:, tb * P:(tb + 1) * P], ident,
                                     is_transpose=True, start=(j == 0), stop=(j == 3),
                                     skip_group_check=True)
                nc.vector.tensor_copy(vv[:, tg * 4:(tg + 1) * 4, dpo * P:(dpo + 1) * P], tp4)

    # ---- text -> TT -> KT, Vv ----
    mark("prepT")
    actT = prep(txt)
    mark("projK")
    proj(wkt, bk_sb, actT, kt, on_vector=False)
    mark("projV")
    proj_v(actT)

    # ---- vision -> VT -> QT ----
    mark("prepV")
    actV = prep(vis)
    mark("projQ")
    proj(wqt, bq_sb, actV, qt, on_vector=True)

    # ---- phase F: scores, softmax, cross_vision (per s-tile) ----
    # Software-pipelined: the scores matmuls of s-tile so+1 are emitted
    # between exp(so) (ACT) and the E-transposes that consume it, so the
    # in-order PE never waits on the ACT engine.
    mark("F")
    e_sb = p_act.tile([P, NT, SEQ], F32R, name="e_sb", tag="act_e")
    rinv = p_rv.tile([P, NT], F32, name="rinv", tag="rinv")
    rps = {}

    def scores_stile(so):
        rp = p_rp.tile([P, 2], F32, name="rp", tag="rp")
        pss = [pp_mm.tile([P, H], F32, name=f"ps_s{i}", tag="mm") for i in range(2)]
        for eo in range(NT):
            for tc_ in range(2):
                nc.tensor.matmul(pss[tc_], qt[:, eo, so * P:(so + 1) * P],
                                 kt[:, eo, tc_ * H:(tc_ + 1) * H],
                                 start=(eo == 0), stop=(eo == NT - 1))
        for tc_ in range(2):
            nc.scalar.activation(out=e_sb[:, so, tc_ * H:(tc_ + 1) * H], in_=pss[tc_],
                                 func=AF.Exp, accum_out=rp[:, tc_:tc_ + 1])
        rps[so] = rp

    scores_stile(0)
    for so in range(NT):
        if so + 1 < NT:
            scores_stile(so + 1)
        rp = rps.pop(so)
        rsum = p_rp.tile([P, 1], F32, name="rsum", tag="rsum")
        nc.vector.tensor_add(rsum, rp[:, 0:1], rp[:, 1:2])
        nc.vector.reciprocal(rinv[:, so:so + 1], rsum)

        # ET blocks for this s-tile (transpose the *unnormalized* E row-block)
        etb = p_etb.tile([P, NT, P], F32R, name="etb", tag="etb")
        for tg in range(2):
            tp4 = pp_t.tile([P, 4, P], F32R, name="tp4e", tag="tp4")
            for j in range(4):
                tt = tg * 4 + j
                nc.tensor.matmul(tp4[:, j, :], e_sb[:, so, tt * P:(tt + 1) * P], ident,
                                 is_transpose=True, start=(j == 0), stop=(j == 3),
                                 skip_group_check=True)
            nc.vector.tensor_copy(etb[:, tg * 4:(tg + 1) * 4, :], tp4)

        # normalize this E row-block in place (for cross_text later)
        nc.vector.tensor_scalar_mul(e_sb[:, so, :], e_sb[:, so, :],
                                    scalar1=rinv[:, so:so + 1])

        # cross_vision[s-tile] = rinv * (ET.T @ Vv)
        cvs = p_cvs.tile([P, DIM], F32, name="cvs", tag="cvs")
        pcv = [pp_mm.tile([P, H], F32, name=f"ps_cv{i}", tag="mm") for i in range(2)]
        for tt in range(NT):
            for dc in range(2):
                nc.tensor.matmul(pcv[dc], etb[:, tt, :], vv[:, tt, dc * H:(dc + 1) * H],
                                 start=(tt == 0), stop=(tt == NT - 1))
        for dc in range(2):
            nc.scalar.mul(cvs[:, dc * H:(dc + 1) * H], pcv[dc], mul=rinv[:, so:so + 1])
        nc.gpsimd.dma_start(out=cv_d[b, so * P:(so + 1) * P, :], in_=cvs)

    # ---- phase H: cross_text = E'.T @ V (E' already rinv-scaled) ----
    # 8 concurrent PSUM accumulation groups (6 from pmm + 2 borrowed from the
    # idle transpose pool): each V tile load feeds 8 matmuls and V is read
    # only once per d'-half. Loads alternate between the two HWDGE queues.
    mark("H")
    for dc in range(2):
        pss = [pp_mm.tile([P, H], F32, name=f"ps_ct{i}", tag="mm") for i in range(6)]
        pss += [pp_t.tile([P, H], F32, name=f"ps_ct{i + 6}", tag="tp4") for i in range(2)]
        for so in range(NT):
            vt = p_vt.tile([P, H], F32R, name="vt", tag="vt")
            eng = nc.sync if so % 2 == 0 else nc.scalar
            eng.dma_start(out=vt, in_=vis[b, so * P:(so + 1) * P, dc * H:(dc + 1) * H].bitcast(F32R))
            for tt in range(NT):
                nc.tensor.matmul(pss[tt], e_sb[:, so, tt * P:(tt + 1) * P], vt,
                                 start=(so == 0), stop=(so == NT - 1))
        for tt in range(NT):
            cts = p_cts.tile([P, H], F32, name="cts", tag="cts")
            if tt % 2 == 0:
                nc.vector.tensor_copy(cts, pss[tt])
            else:
                nc.scalar.copy(cts, pss[tt])
            nc.gpsimd.dma_start(out=ct_d[b, tt * P:(tt + 1) * P, dc * H:(dc + 1) * H],
                                  in_=cts)
    mark("end")


def build_nc():
    nc = bacc.Bacc("TRN2", target_bir_lowering=False, debug=False, num_devices=N_CORES)
    vis = nc.dram_tensor("vision", [BPC, SEQ, DIM], F32, kind="ExternalInput").ap()
    txt = nc.dram_tensor("text", [BPC, SEQ, DIM], F32, kind="ExternalInput").ap()
    wqt = nc.dram_tensor("wqt", [DIM, DIM], F32R, kind="ExternalInput").ap()
    wkt = nc.dram_tensor("wkt", [DIM, DIM], F32R, kind="ExternalInput").ap()
    wvt = nc.dram_tensor("wvt", [DIM, DIM], F32R, kind="ExternalInput").ap()
    bq_d = nc.dram_tensor("bq", [DIM], F32, kind="ExternalInput").ap()
    id_d = nc.dram_tensor("ident128", [P, P], F32R, kind="ExternalInput").ap()
    bk_d = nc.dram_tensor("bk", [DIM], F32, kind="ExternalInput").ap()
    cv_d = nc.dram_tensor("cross_vision", [BPC, SEQ, DIM], F32, kind="ExternalOutput").ap()
    ct_d = nc.dram_tensor("cross_text", [BPC, SEQ, DIM], F32, kind="ExternalOutput").ap()

    with tile.TileContext(nc) as tc:
        pools = []
        import contextlib
        with contextlib.ExitStack() as ctx:
            def sp(name, bufs):
                return ctx.enter_context(tc.tile_pool(name=name, bufs=bufs))

            p_act = sp("act", 1)
            p_kt = sp("kt", 1)
            p_qt = sp("qt", 1)
            p_vv = sp("vv", 1)
            p_etb = sp("etb", 1)
            p_wc = sp("wc", 3)
            p_vvt = sp("vvt", 2)
            p_in = sp("xin", 4)
            p_cvs = sp("cvs", 2)
            p_cts = sp("cts", 4)
            p_vt = sp("vt", 4)
            p_rp = sp("rp", 4)
            p_rv = sp("rv", 2)
            p_sm = sp("sm", 1)
            pp_t = ctx.enter_context(
                tc.tile_pool(name="pp_t", bufs=2, space=bass.MemorySpace.PSUM))
            pp_mm = ctx.enter_context(
                tc.tile_pool(name="pp_mm", bufs=6, space=bass.MemorySpace.PSUM))

            ident = p_sm.tile([P, P], F32R, name="ident")
            nc.sync.dma_start(out=ident, in_=id_d)
            bq_sb = p_sm.tile([P, NT], F32, name="bq_sb")
            nc.sync.dma_start(out=bq_sb, in_=bq_d.rearrange("(eo ei) -> ei eo", ei=P))
            bk_sb = p_sm.tile([P, NT], F32, name="bk_sb")
            nc.sync.dma_start(out=bk_sb, in_=bk_d.rearrange("(eo ei) -> ei eo", ei=P))

            pools = (p_act, p_kt, p_qt, p_vv, p_etb, p_wc, p_vvt, p_in,
                     p_cvs, p_cts, p_vt, p_rp, p_rv, pp_t, pp_mm)
            for b in range(BPC):
                _emit(tc, ident, vis, txt, wqt, wkt, wvt, bq_sb, bk_sb,
                      cv_d, ct_d, pools, b)
    nc.compile()
    return nc


_NC_CACHE = None


def _get_nc():
    global _NC_CACHE
    if _NC_CACHE is None:
        _NC_CACHE = build_nc()
    return _NC_CACHE


def make_in_maps(vision_repr, text_repr, Wq, bq, Wk, bk, Wv, bv):
    s = 1.0 / np.sqrt(np.float32(DIM))
    wqt = np.ascontiguousarray(np.asarray(Wq, np.float32).T * s)
    wkt = np.ascontiguousarray(np.asarray(Wk, np.float32).T)
    wvt = np.ascontiguousarray(np.asarray(Wv, np.float32).T)
    bq_s = np.asarray(bq, np.float32) * s
    bk_ = np.asarray(bk, np.float32)
    vis = np.asarray(vision_repr, np.float32)
    txt = np.asarray(text_repr, np.float32)
    in_maps = []
    for c in range(N_CORES):
        in_maps.append({
            "vision": vis[c * BPC:(c + 1) * BPC],
            "text": txt[c * BPC:(c + 1) * BPC],
            "wqt": wqt, "wkt": wkt, "wvt": wvt,
            "bq": bq_s, "bk": bk_,
            "ident128": np.eye(P, dtype=np.float32),
        })
    return in_maps


def kernel(vision_repr, text_repr, Wq, bq, Wk, bk, Wv, bv):
    from concourse.bass_utils import run_bass_kernel_spmd

    nc = _get_nc()
    in_maps = make_in_maps(vision_repr, text_repr, Wq, bq, Wk, bk, Wv, bv)
    res = run_bass_kernel_spmd(nc, in_maps, list(range(N_CORES))).results
    cv = np.concatenate([r_["cross_vision"] for r_ in res], axis=0)
    ct = np.concatenate([r_["cross_text"] for r_ in res], axis=0)
    cv = cv + np.asarray(bv, np.float32)[None, None, :]
    return cv, ct



# revision 4
# speedup vs baseline: 1.3924x; 1.3924x over previous
"""Cross-attention layer (vision<->text) on 8 Trainium2 NeuronCores.

Problem: B=16, Sv=St=1024, D=1024, fp32.
  q = vision @ Wq.T + bq            [B,Sv,D]
  k = text   @ Wk.T + bk            [B,St,D]
  v = text   @ Wv.T + bv            [B,St,D]
  scores = q @ k.T / sqrt(D)        [B,Sv,St]
  attn = softmax(scores, -1)
  cross_vision = attn @ v           [B,Sv,D]
  cross_text   = attn.T @ vision    [B,St,D]

Sharding: pure data-parallel over batch, 2 items per core, no collectives.

Design (v2 — PE-stream-first):
  - Host stages activations in BOTH orientations as bf16 (visT/txtT [d,s]
    and vis natural [s,d]) plus weights pre-tiled for direct stationary
    loads.  This removes all on-chip PE transposes of the inputs (256 per
    core in v1) and the staging copies that came with them.
  - All matmul operands are bf16 (PSUM accumulation stays fp32).  At
    N=512 the PE streams 1 row/cycle for bf16 and fp32r alike, so this
    costs nothing on the matmul stream but makes everything resident in
    SBUF (181KB/partition), halves DMA, speeds the E-transposes
    (1.0 vs 1.5 cyc/row) and lets LDWEIGHTS use fast-weight-load.
  - Per item: projK -> projV (V computed directly in [t,d] layout with
    text tiles stationary: no transposes) -> projQ -> F (scores/exp/
    E-transpose/cross_vision, software-pipelined one s-tile ahead) ->
    H (cross_text, 16 sequential single-bank PSUM groups).
  - The 1/sqrt(D) is folded into wqt/bq on the host.  bv is added on the
    host after gather (attn rows sum to 1, so attn @ (v0+bv) = attn@v0
    + bv, exact).
  - Weights are loaded once and stay resident across both items; the
    next item's activations prefetch during the current item's F/H so
    the PE never waits at an item boundary (keeps the PE HAM throttle
    at 8/8 — every >3.4us PE idle gap re-throttles the clock to 1.2GHz
    for ~10us).
"""

import sys

import numpy as np

if "/opt/trn_rl_repo" not in sys.path:
    sys.path.insert(0, "/opt/trn_rl_repo")

import ml_dtypes

import concourse.bass as bass
import concourse.tile as tile
from concourse import bacc
from concourse import mybir

P = 128
B, SEQ, DIM = 16, 1024, 1024
N_CORES = 8
BPC = B // N_CORES  # batch items per core
NT = DIM // P  # 8 tiles of 128 along d/e
F32 = mybir.dt.float32
BF16 = mybir.dt.bfloat16
AF = mybir.ActivationFunctionType
H = 512  # half of a seq dim / PSUM-bank-sized chunk
BF_NP = ml_dtypes.bfloat16


def _emit_item(nc, b, tens, pools, cur, nxt):
    """Emit one batch item.  `cur` holds this item's already-loading
    activation tiles (txtT/visT/visn); prefetch tiles for item b+1 are
    allocated here mid-item and returned via `nxt`."""
    (txtT_d, visT_d, visn_d, cv_d, ct_d) = tens["dram"]
    (wq_sb, wk_sb, wv_sb, bq_sb, bk_sb, ident) = tens["wt"]
    (p_txtT, p_visT, p_visn, p_kt, p_qt, p_vv, p_esb, p_etb, p_cvs, p_cts,
     p_rp, p_rv, pp_t, pp_mm) = pools

    txtT, visT, visn = cur["txtT"], cur["visT"], cur["visn"]
    kt = p_kt.tile([P, NT, SEQ], BF16, name="kt", tag="kt")
    qt = p_qt.tile([P, NT, SEQ], BF16, name="qt", tag="qt")
    vv = p_vv.tile([P, NT, SEQ], BF16, name="vv", tag="vv")
    e_sb = p_esb.tile([P, NT, SEQ], BF16, name="e_sb", tag="esb")

    def nxt_loads(phase):
        """Prefetch item b+1 activations once item b is done reading."""
        if b + 1 >= BPC:
            return
        if phase == "projQ":
            # txtT(b) is dead after projV(b); same buffer, sync queue.
            t2 = p_txtT.tile([P, NT, SEQ], BF16, name="txtT", tag="txtT")
            for l in range(NT):
                nc.sync.dma_start(out=t2[:, l, :], in_=txtT_d[b + 1, :, l, :])
            nxt["txtT"] = t2
        elif phase == "F":
            # visT(b) is dead after projQ(b).
            v2 = p_visT.tile([P, NT, SEQ], BF16, name="visT", tag="visT")
            for l in range(NT):
                nc.sync.dma_start(out=v2[:, l, :], in_=visT_d[b + 1, :, l, :])
            nxt["visT"] = v2
            # vis_nat has bufs=2, so this never blocks the gpsimd queue.
            v3 = p_visn.tile([P, NT, SEQ], BF16, name="visn", tag="visn")
            nc.gpsimd.dma_start(out=v3, in_=visn_d[b + 1])
            nxt["visn"] = v3

    # ---- projK: kt[e, t] = sum_do wkt[do,:,e].T @ txtT[do,:,t] (+bk) ----
    def proj(w_sb, bias_col, xT, out_sb):
        for eo in range(NT):
            pss = [pp_mm.tile([P, H], F32, name=f"ps_p{i}", tag="mm")
                   for i in range(2)]
            for do in range(NT):
                for sh in range(2):
                    nc.tensor.matmul(pss[sh], w_sb[:, do, eo, :],
                                     xT[:, do, sh * H:(sh + 1) * H],
                                     start=(do == 0), stop=(do == NT - 1))
            for sh in range(2):
                dst = out_sb[:, eo, sh * H:(sh + 1) * H]
                if sh == 0:
                    nc.vector.tensor_scalar_add(dst, pss[sh],
                                                scalar1=bias_col[:, eo:eo + 1])
                else:
                    nc.scalar.add(dst, pss[sh], add=bias_col[:, eo:eo + 1])

    proj(wk_sb, bk_sb, txtT, kt)

    # ---- projV: vv[t, d] = sum_do txtT[do,:,t-block].T @ wvt[do,:,d] ----
    for tb in range(NT):
        pss = [pp_mm.tile([P, H], F32, name=f"ps_v{i}", tag="mm")
               for i in range(2)]
        for do in range(NT):
            for dh in range(2):
                nc.tensor.matmul(pss[dh], txtT[:, do, tb * P:(tb + 1) * P],
                                 wv_sb[:, do, dh * H:(dh + 1) * H],
                                 start=(do == 0), stop=(do == NT - 1))
        for dh in range(2):
            dst = vv[:, tb, dh * H:(dh + 1) * H]
            if dh == 0:
                nc.vector.tensor_copy(dst, pss[dh])
            else:
                nc.scalar.copy(dst, pss[dh])

    # ---- projQ ----
    nxt_loads("projQ")
    proj(wq_sb, bq_sb, visT, qt)

    # ---- phase F: scores, softmax, cross_vision (per s-tile) ----
    # Software-pipelined: the scores matmuls of s-tile so+1 are emitted
    # between exp(so) (ACT) and the E-transposes that consume it, so the
    # in-order PE never waits on the ACT engine.
    nxt_loads("F")
    rinv = p_rv.tile([P, NT], F32, name="rinv", tag="rinv")
    rps = {}

    def scores_stile(so):
        rp = p_rp.tile([P, 2], F32, name="rp", tag="rp")
        pss = [pp_mm.tile([P, H], F32, name=f"ps_s{i}", tag="mm")
               for i in range(2)]
        for eo in range(NT):
            for th in range(2):
                nc.tensor.matmul(pss[th], qt[:, eo, so * P:(so + 1) * P],
                                 kt[:, eo, th * H:(th + 1) * H],
                                 start=(eo == 0), stop=(eo == NT - 1))
        for th in range(2):
            nc.scalar.activation(out=e_sb[:, so, th * H:(th + 1) * H],
                                 in_=pss[th], func=AF.Exp,
                                 accum_out=rp[:, th:th + 1])
        rps[so] = rp

    scores_stile(0)
    for so in range(NT):
        if so + 1 < NT:
            scores_stile(so + 1)
        rp = rps.pop(so)
        rsum = p_rp.tile([P, 1], F32, name="rsum", tag="rsum")
        nc.vector.tensor_add(rsum, rp[:, 0:1], rp[:, 1:2])
        nc.vector.reciprocal(rinv[:, so:so + 1], rsum)

        # ET blocks for this s-tile (transpose the *unnormalized* E row-block)
        etb = p_etb.tile([P, NT, P], BF16, name="etb", tag="etb")
        for tg in range(2):
            tp4 = pp_t.tile([P, 4, P], BF16, name="tp4e", tag="tp4")
            for j in range(4):
                tt = tg * 4 + j
                nc.tensor.matmul(tp4[:, j, :], e_sb[:, so, tt * P:(tt + 1) * P],
                                 ident, is_transpose=True,
                                 start=(j == 0), stop=(j == 3),
                                 skip_group_check=True)
            nc.vector.tensor_copy(etb[:, tg * 4:(tg + 1) * 4, :], tp4)

        # normalize this E row-block in place (for cross_text later)
        nc.vector.tensor_scalar_mul(e_sb[:, so, :], e_sb[:, so, :],
                                    scalar1=rinv[:, so:so + 1])

        # cross_vision[s-tile] = rinv * (ET.T @ Vv)
        cvs = p_cvs.tile([P, DIM], F32, name="cvs", tag="cvs")
        pcv = [pp_mm.tile([P, H], F32, name=f"ps_cv{i}", tag="mm")
               for i in range(2)]
        for tt in range(NT):
            for dh in range(2):
                nc.tensor.matmul(pcv[dh], etb[:, tt, :],
                                 vv[:, tt, dh * H:(dh + 1) * H],
                                 start=(tt == 0), stop=(tt == NT - 1))
        for dh in range(2):
            nc.scalar.mul(cvs[:, dh * H:(dh + 1) * H], pcv[dh],
                          mul=rinv[:, so:so + 1])
        nc.scalar.dma_start(out=cv_d[b, so * P:(so + 1) * P, :], in_=cvs)

    # ---- phase H: cross_text = E'.T @ vis (E' already rinv-scaled) ----
    # 16 sequential single-bank PSUM groups; vis natural is SBUF-resident
    # so the 128 matmuls stream back-to-back with no DMA dependence.
    for dh in range(2):
        for tt in range(NT):
            ps = pp_mm.tile([P, H], F32, name="ps_ct", tag="mm")
            for so in range(NT):
                nc.tensor.matmul(ps, e_sb[:, so, tt * P:(tt + 1) * P],
                                 visn[:, so, dh * H:(dh + 1) * H],
                                 start=(so == 0), stop=(so == NT - 1))
            cts = p_cts.tile([P, H], F32, name="cts", tag="cts")
            if tt % 2 == 0:
                nc.vector.tensor_copy(cts, ps)
            else:
                nc.scalar.copy(cts, ps)
            nc.gpsimd.dma_start(
                out=ct_d[b, tt * P:(tt + 1) * P, dh * H:(dh + 1) * H], in_=cts)


def build_nc():
    nc = bacc.Bacc("TRN2", target_bir_lowering=False, debug=False,
                   num_devices=N_CORES)
    txtT_d = nc.dram_tensor("txtT", [BPC, P, NT, SEQ], BF16,
                            kind="ExternalInput").ap()
    visT_d = nc.dram_tensor("visT", [BPC, P, NT, SEQ], BF16,
                            kind="ExternalInput").ap()
    visn_d = nc.dram_tensor("visn", [BPC, P, NT, SEQ], BF16,
                            kind="ExternalInput").ap()
    wq_d = nc.dram_tensor("wq", [P, NT, NT, P], BF16, kind="ExternalInput").ap()
    wk_d = nc.dram_tensor("wk", [P, NT, NT, P], BF16, kind="ExternalInput").ap()
    wv_d = nc.dram_tensor("wv", [P, NT, SEQ], BF16, kind="ExternalInput").ap()
    bq_d = nc.dram_tensor("bq", [P, NT], F32, kind="ExternalInput").ap()
    bk_d = nc.dram_tensor("bk", [P, NT], F32, kind="ExternalInput").ap()
    id_d = nc.dram_tensor("ident128", [P, P], BF16, kind="ExternalInput").ap()
    cv_d = nc.dram_tensor("cross_vision", [BPC, SEQ, DIM], F32,
                          kind="ExternalOutput").ap()
    ct_d = nc.dram_tensor("cross_text", [BPC, SEQ, DIM], F32,
                          kind="ExternalOutput").ap()

    with tile.TileContext(nc) as tc:
        import contextlib
        with contextlib.ExitStack() as ctx:
            def sp(name, bufs):
                return ctx.enter_context(tc.tile_pool(name=name, bufs=bufs))

            p_wt = sp("wt", 1)
            p_txtT = sp("txtT", 1)
            p_visT = sp("visT", 1)
            p_visn = sp("visn", 2)
            p_kt = sp("kt", 1)
            p_qt = sp("qt", 1)
            p_vv = sp("vv", 1)
            p_esb = sp("esb", 1)
            p_etb = sp("etb", 2)
            p_cvs = sp("cvs", 2)
            p_cts = sp("cts", 4)
            p_rp = sp("rp", 4)
            p_rv = sp("rv", 2)
            pp_t = ctx.enter_context(
                tc.tile_pool(name="pp_t", bufs=2, space=bass.MemorySpace.PSUM))
            pp_mm = ctx.enter_context(
                tc.tile_pool(name="pp_mm", bufs=6, space=bass.MemorySpace.PSUM))

            # small constants first on sync (needed at first proj evac)
            ident = p_wt.tile([P, P], BF16, name="ident")
            nc.sync.dma_start(out=ident, in_=id_d)
            bq_sb = p_wt.tile([P, NT], F32, name="bq_sb")
            nc.sync.dma_start(out=bq_sb, in_=bq_d)
            bk_sb = p_wt.tile([P, NT], F32, name="bk_sb")
            nc.sync.dma_start(out=bk_sb, in_=bk_d)

            # resident weights on gpsimd, chunked by do, in use order (K,V,Q)
            wk_sb = p_wt.tile([P, NT, NT, P], BF16, name="wk_sb")
            for do in range(NT):
                nc.gpsimd.dma_start(out=wk_sb[:, do, :, :], in_=wk_d[:, do, :, :])
            wv_sb = p_wt.tile([P, NT, SEQ], BF16, name="wv_sb")
            for do in range(NT):
                nc.gpsimd.dma_start(out=wv_sb[:, do, :], in_=wv_d[:, do, :])
            wq_sb = p_wt.tile([P, NT, NT, P], BF16, name="wq_sb")
            for do in range(NT):
                nc.gpsimd.dma_start(out=wq_sb[:, do, :, :], in_=wq_d[:, do, :, :])

            tens = {
                "dram": (txtT_d, visT_d, visn_d, cv_d, ct_d),
                "wt": (wq_sb, wk_sb, wv_sb, bq_sb, bk_sb, ident),
            }
            pools = (p_txtT, p_visT, p_visn, p_kt, p_qt, p_vv, p_esb, p_etb,
                     p_cvs, p_cts, p_rp, p_rv, pp_t, pp_mm)

            # item-0 activation loads (chunked so the first matmuls can
            # start before the full tensors land): txtT then visT on sync,
            # vis natural on gpsimd behind the weights.
            cur = {}
            cur["txtT"] = p_txtT.tile([P, NT, SEQ], BF16, name="txtT",
                                      tag="txtT")
            for l in range(NT):
                nc.sync.dma_start(out=cur["txtT"][:, l, :],
                                  in_=txtT_d[0, :, l, :])
            cur["visT"] = p_visT.tile([P, NT, SEQ], BF16, name="visT",
                                      tag="visT")
            for l in range(NT):
                nc.sync.dma_start(out=cur["visT"][:, l, :],
                                  in_=visT_d[0, :, l, :])
            cur["visn"] = p_visn.tile([P, NT, SEQ], BF16, name="visn",
                                      tag="visn")
            nc.gpsimd.dma_start(out=cur["visn"], in_=visn_d[0])

            for b in range(BPC):
                nxt = {}
                _emit_item(nc, b, tens, pools, cur, nxt)
                cur = nxt
    nc.compile()
    return nc


_NC_CACHE = None


def _get_nc():
    global _NC_CACHE
    if _NC_CACHE is None:
        _NC_CACHE = build_nc()
    return _NC_CACHE


def make_in_maps(vision_repr, text_repr, Wq, bq, Wk, bk, Wv, bv):
    s = 1.0 / np.sqrt(np.float32(DIM))

    def wtile(w, scale=None):
        wt = np.asarray(w, np.float32).T
        if scale is not None:
            wt = wt * scale
        # [d, e] -> [di, do, eo, ei]
        return np.ascontiguousarray(
            wt.reshape(NT, P, NT, P).transpose(1, 0, 2, 3)).astype(BF_NP)

    wq_t = wtile(Wq, s)
    wk_t = wtile(Wk)
    wv_t = np.ascontiguousarray(
        np.asarray(Wv, np.float32).T.reshape(NT, P, SEQ).transpose(1, 0, 2)
    ).astype(BF_NP)
    bq_s = np.ascontiguousarray(
        (np.asarray(bq, np.float32) * s).reshape(NT, P).T)
    bk_s = np.ascontiguousarray(np.asarray(bk, np.float32).reshape(NT, P).T)

    vis = np.asarray(vision_repr, np.float32)
    txt = np.asarray(text_repr, np.float32)
    # [b, s, d] -> transposed [b, di, l, s] and natural [b, si, so, d]
    visT = np.ascontiguousarray(
        vis.transpose(0, 2, 1).reshape(B, NT, P, SEQ).transpose(0, 2, 1, 3)
    ).astype(BF_NP)
    txtT = np.ascontiguousarray(
        txt.transpose(0, 2, 1).reshape(B, NT, P, SEQ).transpose(0, 2, 1, 3)
    ).astype(BF_NP)
    visn = np.ascontiguousarray(
        vis.reshape(B, NT, P, DIM).transpose(0, 2, 1, 3)).astype(BF_NP)

    ident = np.eye(P, dtype=BF_NP)
    in_maps = []
    for c in range(N_CORES):
        sl = slice(c * BPC, (c + 1) * BPC)
        in_maps.append({
            "txtT": txtT[sl], "visT": visT[sl], "visn": visn[sl],
            "wq": wq_t, "wk": wk_t, "wv": wv_t,
            "bq": bq_s, "bk": bk_s,
            "ident128": ident,
        })
    return in_maps


def kernel(vision_repr, text_repr, Wq, bq, Wk, bk, Wv, bv):
    from concourse.bass_utils import run_bass_kernel_spmd

    nc = _get_nc()
    in_maps = make_in_maps(vision_repr, text_repr, Wq, bq, Wk, bk, Wv, bv)
    res = run_bass_kernel_spmd(nc, in_maps, list(range(N_CORES))).results
    cv = np.concatenate([r_["cross_vision"] for r_ in res], axis=0)
    ct = np.concatenate([r_["cross_text"] for r_ in res], axis=0)
    cv = cv + np.asarray(bv, np.float32)[None, None, :]
    return cv, ct
